# revision 63
# baseline (speedup 1.0000x reference)
"""DigitCaps routing kernel for Trainium2 (8 NeuronCores, SPMD data-parallel over batch).

Math (per batch element b):
  u_hat[r, c, o] = sum_i W[r, c, o, i] * x[r, i]
  b_log = 0
  repeat 3x:
    c = softmax(b_log, axis=c)
    s[c, o] = sum_r c[r, c] * u_hat[r, c, o]
    v = squash(s)                               (over o)
    b_log += sum_o u_hat[r, c, o] * v[c, o]     (first 2 iters only)
  return v

Layout strategy per core (B_local = 32 batches):
  - r split into 72 supergroups (rg) of 16 (r16); phase A contracts K =
    (r16, i) = 128 using a host-built block-diagonal x as the stationary
    operand: lhsT[(r16, i), (r16', b8)] = x[b, r, i] * delta(r16, r16').
  - u_hat lives in SBUF as bf16 [(r16, b8)=128 partitions, (bg=4, rg=72, c=11, o=16)].
  - s-step: y = c_sm (.) u_hat (free-dim broadcast over o) on DVE, then the
    r16-partition sum via a constant block-diagonal-ones stationary matmul.
  - agreement: z = u_hat (.) v_t, then o-group tensor_reduce on DVE; v_t is v
    replicated across the 16 r16 partition groups via a tiny PE matmul with a
    constant 0/1 replication matrix.
"""

import math
import sys
for p in ("/opt/trn_rl_repo", "/root/.axon_site/_ro/trn_rl_repo"):
    if p not in sys.path:
        sys.path.insert(0, p)

import numpy as np
import ml_dtypes
from contextlib import ExitStack

import concourse.bass as bass
import concourse.tile as tile
from concourse import bacc, mybir
from concourse.bass_utils import run_bass_kernel_spmd

# problem constants
B, R, C, I, O = 256, 1152, 11, 8, 16
ITERS = 3
EPS = 1e-9

N_CORES = 8
BL = B // N_CORES          # 32 batches per core
RG = R // 16               # 72 supergroups of 16 r
CO = C * O                 # 176
NBG = BL // 8              # 4 b-groups of 8
NT = RG * NBG              # 288 phase-A tiles
STG = 8                    # phase-A tiles per staging DMA
CH = 8                     # rg chunk for premultiplies
F32 = mybir.dt.float32
BF16 = mybir.dt.bfloat16
BF16_NP = ml_dtypes.bfloat16


def _build_program(reps=1, abl=0):
    nc = bacc.Bacc("TRN2", target_bir_lowering=False, debug=False, num_devices=N_CORES)

    xbd_d = nc.dram_tensor("xbd", [NT // STG, 128, STG * 128], BF16, kind="ExternalInput").ap()
    wt_d = nc.dram_tensor("wt", [128, RG * CO], BF16, kind="ExternalInput").ap()
    xc_d = nc.dram_tensor("xc", [128, RG * BL], BF16, kind="ExternalInput").ap()
    ones_d = nc.dram_tensor("onesbd", [NBG, 128, 32], BF16, kind="ExternalInput").ap()
    rep_d = nc.dram_tensor("rep", [NBG, 32, 128], F32, kind="ExternalInput").ap()
    vout_d = nc.dram_tensor("vout", [BL, CO], F32, kind="ExternalOutput").ap()

    with tile.TileContext(nc) as tc, ExitStack() as ctx:
        const_p = ctx.enter_context(tc.tile_pool(name="const", bufs=1))
        stg_p = ctx.enter_context(tc.tile_pool(name="stg", bufs=3))
        uh_p = ctx.enter_context(tc.tile_pool(name="uh", bufs=1))
        log_p = ctx.enter_context(tc.tile_pool(name="logit", bufs=1))
        sm_p = ctx.enter_context(tc.tile_pool(name="smx", bufs=1))
        y_p = ctx.enter_context(tc.tile_pool(name="y", bufs=3))
        a_p = ctx.enter_context(tc.tile_pool(name="a", bufs=2))
        f1_p = ctx.enter_context(tc.tile_pool(name="f1p", bufs=2))
        f2_p = ctx.enter_context(tc.tile_pool(name="f2p", bufs=2))
        f3_p = ctx.enter_context(tc.tile_pool(name="f3p", bufs=2))
        vt_p = ctx.enter_context(tc.tile_pool(name="vt", bufs=2))
        sq_p = ctx.enter_context(tc.tile_pool(name="sq", bufs=2))
        psA = ctx.enter_context(tc.tile_pool(name="psA", bufs=3, space=bass.MemorySpace.PSUM))
        psS = ctx.enter_context(tc.tile_pool(name="psS", bufs=2, space=bass.MemorySpace.PSUM))

        w_sb = const_p.tile([128, RG * CO], BF16)
        WCH = 9
        for wc in range(0, RG, WCH):
            nc.sync.dma_start(
                w_sb[:, wc * CO:(wc + WCH) * CO], wt_d[:, wc * CO:(wc + WCH) * CO]
            )
        xc_sb = const_p.tile([128, RG * BL], BF16)
        nc.sync.dma_start(xc_sb[:], xc_d[:])
        ones_sb = const_p.tile([128, NBG * 32], BF16)
        nc.sync.dma_start(
            ones_sb[:].rearrange("p (g m) -> p g m", g=NBG),
            ones_d[:].transpose([1, 0, 2]),
        )
        ones3 = ones_sb[:].rearrange("p (g m) -> p g m", g=NBG)
        rep_sb = const_p.tile([32, NBG * 128], F32)
        nc.sync.dma_start(
            rep_sb[:].rearrange("p (g m) -> p g m", g=NBG),
            rep_d[:].transpose([1, 0, 2]),
        )
        eps_t = const_p.tile([32, 1], F32)
        nc.vector.memset(eps_t[:], EPS)
        one_t = const_p.tile([32, 1], F32)
        nc.vector.memset(one_t[:], 1.0)
        lnc_t = const_p.tile([32, 1], F32)
        nc.vector.memset(lnc_t[:], math.log(1.0 / C))
        zero_t = const_p.tile([32, 1], F32)
        nc.vector.memset(zero_t[:], 0.0)

        def emit_body():
            # u_hat free layout: (bg, rg, c, o)
            u_hat = uh_p.tile([128, NBG * RG * CO], BF16)
            uh4 = u_hat[:].rearrange("p (g r f) -> p g r f", g=NBG, r=RG)

            # logits & softmax buffers (f32)
            b_log = log_p.tile([128, NBG * RG * C], BF16)
            bl3 = b_log[:].rearrange("p (g r c) -> p g r c", g=NBG, r=RG)
            rsum = sm_p.tile([128, NBG * RG], F32)
            rrec = sm_p.tile([128, NBG * RG], F32)
            c_sm = sm_p.tile([128, NBG * RG * C * 2], BF16)  # pair-duplicated
            cs4 = c_sm[:].rearrange("p (g r c t) -> p g r c t", g=NBG, r=RG, c=C)

            s_sb = sq_p.tile([32, CO], F32)
            sqv = sq_p.tile([32, CO], F32)
            ss = sq_p.tile([32, C], F32)
            t2 = sq_p.tile([32, C], F32)
            sqr = sq_p.tile([32, C], F32)
            den = sq_p.tile([32, C], F32)
            rf = sq_p.tile([32, C], F32)
            fac = sq_p.tile([32, C], F32)
            v_sb = sq_p.tile([32, CO], F32)

            def squash(scale):
                # v_sb = squash(s_sb * scale) over o, via ln/exp only:
                # fac = exp(0.5*ln(q+eps) - ln(1+q) + ln(scale)), q = scale^2*ss
                s2 = scale * scale
                nc.vector.tensor_mul(sqv[:], s_sb[:], s_sb[:])
                nc.vector.tensor_reduce(
                    ss[:], sqv[:].rearrange("p (c o) -> p c o", c=C),
                    axis=mybir.AxisListType.X, op=mybir.AluOpType.add,
                )
                nc.scalar.activation(sqr[:], ss[:], mybir.ActivationFunctionType.Ln,
                                     bias=eps_t[:], scale=s2)
                nc.scalar.activation(t2[:], ss[:], mybir.ActivationFunctionType.Ln,
                                     bias=one_t[:], scale=s2)
                nc.vector.scalar_tensor_tensor(
                    den[:], sqr[:], 0.5, t2[:],
                    mybir.AluOpType.mult, mybir.AluOpType.subtract,
                )
                nc.scalar.activation(fac[:], den[:], mybir.ActivationFunctionType.Exp,
                                     bias=(lnc_t[:] if scale != 1.0 else zero_t[:]))
                nc.vector.tensor_mul(
                    v_sb[:].rearrange("p (c o) -> p c o", c=C),
                    s_sb[:].rearrange("p (c o) -> p c o", c=C),
                    fac[:].unsqueeze(2).broadcast_to([32, C, O]),
                )

            def replicate_v():
                # vt_bg[(r16, b8), co] = v[bg*8 + b8, co], via PE with the
                # constant 0/1 replication matrix (f32 matmul, K=32).
                vts = []
                for bg in range(NBG):
                    ps = psA.tile([128, 1024], F32, tag="ps")
                    nc.tensor.matmul(
                        ps[:, :CO],
                        rep_sb[:].rearrange("p (g m) -> p g m", g=NBG)[:, bg, :],
                        v_sb[:],
                        start=True, stop=True,
                    )
                    vt = vt_p.tile([128, CO], BF16, tag=f"vt{bg}")
                    nc.scalar.activation(vt[:], ps[:, :CO],
                                         mybir.ActivationFunctionType.Copy)
                    vts.append(vt)
                return vts

            ACH = 24  # agreement rg chunk (bigger ops amortize DVE overhead)

            def agree_chunk(vts, bg, r0, first):
                # b_log (+)= sum_o u_hat * v_t; the o-sum is a bf16 fold tree
                # (tensor_tensor adds stay in DVE 2x mode, unlike the 1x-rate
                # X-axis tensor_reduce)
                z = y_p.tile([128, ACH * CO], BF16, tag="y")
                nc.vector.tensor_mul(
                    z[:].rearrange("p (r f) -> p r f", r=ACH),
                    uh4[:, bg, r0:r0 + ACH, :],
                    vts[bg][:].unsqueeze(1).broadcast_to([128, ACH, CO]),
                )
                dst = bl3[:, bg, r0:r0 + ACH, :]
                zv = z[:].rearrange("p (r c o) -> p r c o", r=ACH, c=C)
                f1 = f1_p.tile([128, ACH * C * 8], BF16, tag="f1")
                f1v = f1[:].rearrange("p (r c o) -> p r c o", r=ACH, c=C)
                nc.vector.tensor_add(f1v, zv[:, :, :, 0:8], zv[:, :, :, 8:16])
                f2 = f2_p.tile([128, ACH * C * 4], BF16, tag="f2")
                f2v = f2[:].rearrange("p (r c o) -> p r c o", r=ACH, c=C)
                nc.vector.tensor_add(f2v, f1v[:, :, :, 0:4], f1v[:, :, :, 4:8])
                f3 = f3_p.tile([128, ACH * C * 2], BF16, tag="f3")
                f3v = f3[:].rearrange("p (r c o) -> p r c o", r=ACH, c=C)
                nc.vector.tensor_add(f3v, f2v[:, :, :, 0:2], f2v[:, :, :, 2:4])
                if first:
                    nc.vector.tensor_add(dst, f3v[:, :, :, 0], f3v[:, :, :, 1])
                else:
                    ac = a_p.tile([128, ACH * C], BF16, tag="ac")
                    acv = ac[:].rearrange("p (r c) -> p r c", r=ACH)
                    nc.vector.tensor_add(acv, f3v[:, :, :, 0], f3v[:, :, :, 1])
                    nc.vector.tensor_add(dst, dst, acv)

            def agreement(vts, first):
                for bg in range(NBG):
                    for r0 in range(0, RG, ACH):
                        agree_chunk(vts, bg, r0, first)

            rs3 = rsum[:].rearrange("p (g r) -> p g r", g=NBG)
            rr3 = rrec[:].rearrange("p (g r) -> p g r", g=NBG)

            def softmax_bg(bg):
                # softmax with the normalization folded into the exponent:
                # c = exp(b - ln(rsum)). The unnormalized exp lands in the
                # even lanes of c_sm (for the row-sum only); the final exp
                # writes the pair-duplicated bf16 coefficients directly,
                # replacing the recip+dup-copy+normalize-mul DVE chain with
                # one subtract.
                nc.scalar.activation(
                    cs4[:, bg, :, :, 0], bl3[:, bg],
                    mybir.ActivationFunctionType.Exp,
                )
                nc.vector.tensor_reduce(
                    rs3[:, bg], cs4[:, bg, :, :, 0],
                    axis=mybir.AxisListType.X, op=mybir.AluOpType.add,
                )
                nc.scalar.activation(rr3[:, bg], rs3[:, bg],
                                     mybir.ActivationFunctionType.Ln)
                bs = f1_p.tile([128, ACH * C * 4], F32, tag="f1")
                bsv = bs[:, :RG * C].rearrange("p (r c) -> p r c", r=RG)
                nc.vector.tensor_sub(
                    bsv, bl3[:, bg],
                    rr3[:, bg].unsqueeze(2).broadcast_to([128, RG, C]),
                )
                nc.scalar.activation(
                    cs4[:, bg],
                    bsv.unsqueeze(3).broadcast_to([128, RG, C, 2]),
                    mybir.ActivationFunctionType.Exp,
                )

            SCH = 24

            def sstep_bg(ps, bg):
                # s (+)= sum_r c_sm * u_hat for this bg: DVE premultiply
                # (pair-duplicated coeffs keep DVE in 2x mode) +
                # block-diag-ones matmuls (2-rg fold into [32, 2*CO] PSUM).
                for r0 in range(0, RG, SCH):
                    y = y_p.tile([128, SCH * CO], BF16, tag="y")
                    nc.vector.tensor_mul(
                        y[:].rearrange("p (g q t) -> p g q t", q=O // 2, t=2),
                        uh4[:, bg, r0:r0 + SCH, :].rearrange(
                            "p r (g q t) -> p (r g) q t", g=C, t=2),
                        cs4[:, bg, r0:r0 + SCH, :, :].rearrange(
                            "p r c t -> p (r c) t").unsqueeze(2)
                            .broadcast_to([128, SCH * C, O // 2, 2]),
                    )
                    for j in range(0, SCH, 2):
                        rg = r0 + j
                        nc.tensor.matmul(
                            ps[:], ones3[:, bg, :], y[:, j * CO:(j + 2) * CO],
                            start=(bg == 0 and rg == 0),
                            stop=(bg == NBG - 1 and rg == RG - 2),
                        )

            def s_step():
                ps = psS.tile([32, 2 * CO], F32, tag="psS")
                for bg in range(NBG):
                    softmax_bg(bg)
                    sstep_bg(ps, bg)
                nc.vector.tensor_copy(s_sb[:], ps[:, :CO])
                nc.vector.tensor_add(s_sb[:], s_sb[:], ps[:, CO:])

            if abl == 2:
                nc.vector.memset(b_log[:], 0.0)

            # phase A tile body: 4 matmuls (one per bg) land in a 2-bank psum
            # tile at offsets 0/176/512/688; one batched drain per rg writes
            # all 4 slices. Drains split: DVE takes half of block 0 (it is
            # idle before its first agreement chunk), ACT gets the rest.
            PSOFF = (0, 176, 512, 688)

            def phase_a_tile(t):
                stg = stg_p.tile([128, STG * 128], BF16)
                nc.sync.dma_start(stg[:], xbd_d[t])
                for rg2 in range(2):
                    rg = t * 2 + rg2
                    ps = psA.tile([128, 1024], F32, tag="ps")
                    for bg in range(NBG):
                        k = rg2 * NBG + bg
                        nc.tensor.matmul(
                            ps[:, PSOFF[bg]:PSOFF[bg] + CO],
                            stg[:, k * 128:(k + 1) * 128],
                            w_sb[:, rg * CO:(rg + 1) * CO],
                            start=True, stop=True,
                        )
                    src_ap = ps[:].rearrange("p (a q) -> p a q", a=2)[:, :, :2 * CO] \
                        .rearrange("p a (b f) -> p a b f", b=2)
                    dst = uh4[:, :, rg, :].rearrange("p (a b) f -> p a b f", a=2)
                    if t < 9 and rg % 2 == 1:
                        nc.vector.tensor_copy(dst, src_ap)
                    else:
                        nc.scalar.activation(dst, src_ap, mybir.ActivationFunctionType.Copy)

            # ---------------- phase A block 0 ----------------
            # Emitted FIRST so that in the steady state of the timed loop the
            # next rep's u_hat production (PE matmuls + drains) starts right
            # behind this rep's PE tail instead of stalling behind s0+squash.
            for t in range(9):
                phase_a_tile(t)

            # ---------------- iter 0 head: s0 from compact x ----------------
            # c is uniform (1/11) in iter 0, so s0 = (1/11) sum_r u_hat can be
            # computed directly from x and W before u_hat exists; this lets
            # iter-0's agreement (DVE) overlap phase A (PE/ACT/DMA).
            ps0 = psS.tile([32, 2 * CO], F32, tag="psS")
            for rg in range(RG):
                nc.tensor.matmul(
                    ps0[:, :CO], xc_sb[:, rg * BL:(rg + 1) * BL],
                    w_sb[:, rg * CO:(rg + 1) * CO],
                    start=(rg == 0), stop=(rg == RG - 1),
                )

            nc.scalar.activation(s_sb[:], ps0[:, :CO], mybir.ActivationFunctionType.Copy)
            squash(1.0 / C)
            vts0 = replicate_v()
            for bg in range(NBG):
                agree_chunk(vts0, bg, 0, first=True)

            # ------------- phase A blocks 1-3 (+ interleaved agreement) -----
            for t in range(9, NT // STG):
                phase_a_tile(t)
                if t % 12 == 11:
                    r0a = (t // 12) * ACH
                    for bg in range(NBG):
                        agree_chunk(vts0, bg, r0a, first=True)

            if abl != 1:
                # ---------------- iter 1 ----------------
                s_step()
                squash(1.0)
                if abl == 0:
                    vts = replicate_v()
                    agreement(vts, first=False)

                # ---------------- iter 2 ----------------
                s_step()
                squash(1.0)
            nc.sync.dma_start(vout_d[:], v_sb[:])

        # Timed-loop emission: unroll several bodies per For_i iteration so
        # the per-iteration back-edge overhead (~19us measured) amortizes and
        # consecutive reps pipeline without an intervening branch.
        if reps == 1:
            emit_body()
        else:
            unroll = 16 if reps % 16 == 0 else (2 if reps % 2 == 0 else 1)
            with tc.For_i(0, reps // unroll, 1):
                for _ in range(unroll):
                    emit_body()

    # Build-time hint for the activation-table pass: this kernel only uses
    # Ln/Exp/Copy, all present in the (real) natural_log_exp_and_others set.
    # The pass greedily picks the first set containing each function, which
    # splits Ln and Exp across two sets and reloads tables at every
    # squash/softmax transition (~1.3us each, inside the timed loop). Prune
    # ln/exp from the other sets (names and order preserved, so emitted
    # act_func_set_ids stay valid act_info.json indices) so one set serves
    # the whole program and the single load hoists out of the loop.
    import concourse.bacc as _bacc_mod
    _orig_tables = _bacc_mod.get_activation_tables

    def _patched_tables(arch):
        out = {}
        for name, funcs in _orig_tables(arch).items():
            funcs = set(funcs)
            if name != "natural_log_exp_and_others":
                funcs.discard(mybir.ActivationFunctionType.Exp)
                funcs.discard(mybir.ActivationFunctionType.Ln)
            out[name] = funcs
        return out

    _bacc_mod.get_activation_tables = _patched_tables
    try:
        nc.compile()
    finally:
        _bacc_mod.get_activation_tables = _orig_tables
    return nc


_CACHE = {}


def _get_program():
    if "nc" not in _CACHE:
        _CACHE["nc"] = _build_program()
    return _CACHE["nc"]


def _host_xbd(x_l):
    """Block-diag x, staged for DMA: [NT//STG, 128, STG*128] bf16."""
    xr = x_l.reshape(BL, RG, 16, I)
    xbd = np.zeros((NT, 128, 128), dtype=BF16_NP)
    blk = xbd.reshape(RG, NBG, 128, 128)
    for r16 in range(16):
        t = xr[:, :, r16, :]                                 # [BL, RG, I]
        t = t.transpose(1, 2, 0)                             # [RG, I, BL]
        t = t.reshape(RG, I, NBG, 8).transpose(0, 2, 1, 3)   # [RG, NBG, I, 8]
        blk[:, :, r16 * 8:(r16 + 1) * 8, r16 * 8:(r16 + 1) * 8] = t.astype(BF16_NP)
    return np.ascontiguousarray(
        xbd.reshape(NT // STG, STG, 128, 128).transpose(0, 2, 1, 3)
        .reshape(NT // STG, 128, STG * 128)
    )


def _make_in_maps(x, W):
    x = np.asarray(x, dtype=np.float32)
    W = np.asarray(W, dtype=np.float32)

    wt = np.ascontiguousarray(
        W.reshape(RG, 16, C, O, I).transpose(1, 4, 0, 2, 3).reshape(128, RG * CO)
    ).astype(BF16_NP)
    ones_bd = np.zeros((NBG, 128, 32), dtype=BF16_NP)
    for bg in range(NBG):
        for p in range(128):
            ones_bd[bg, p, bg * 8 + p % 8] = 1.0
    rep = np.zeros((NBG, 32, 128), dtype=np.float32)
    for bg in range(NBG):
        for r16 in range(16):
            for b8 in range(8):
                rep[bg, bg * 8 + b8, r16 * 8 + b8] = 1.0

    in_maps = []
    for core in range(N_CORES):
        x_l = x[core * BL:(core + 1) * BL]
        xc = np.ascontiguousarray(
            x_l.reshape(BL, RG, 16, I).transpose(2, 3, 1, 0).reshape(128, RG * BL)
        ).astype(BF16_NP)
        in_maps.append({
            "xbd": _host_xbd(x_l),
            "wt": wt,
            "xc": xc,
            "onesbd": ones_bd,
            "rep": rep,
        })
    return in_maps


def kernel(x, W):
    in_maps = _make_in_maps(x, W)
    nc = _get_program()
    res = run_bass_kernel_spmd(nc, in_maps, list(range(N_CORES)))
    out = np.concatenate(
        [res.results[i]["vout"].reshape(BL, C, O) for i in range(N_CORES)], axis=0
    )
    return out.astype(np.float32)


if __name__ == "__main__":
    rng = np.random.default_rng(0)
    x = rng.standard_normal((B, R, I), dtype=np.float32)
    W = (rng.standard_normal((R, C, O, I), dtype=np.float32) * 0.01).astype(np.float32)
    v = kernel(x=x, W=W)
    print("out", v.shape, v.dtype, np.abs(v).mean())
